# revision 13
# baseline (speedup 1.0000x reference)
"""Balanced CE loss + accuracy on 8 Trainium2 NeuronCores (Bass/Tile).

Reference computation (N = 16777216 elements):
    loss = -sum(where(t==1, 1.6*log(p), 0.4*log(1-p))) / N
    acc  = mean(round(p) == t)

Strategy (data-parallel over N, no collectives):
  Shard N across 8 cores; per core stream [128, C] sub-chunks.  The
  kernel's floor is the irreducible HBM traffic (8 B/elem, ~41-47 us
  per core), so the per-element compute is spread over four engines
  until each is comfortably below that floor:

    ACT   u = ln(p) -> bf16 and v = ln(1-p) -> bf16 (scale=-1, bias=1;
          accum gives sum(v)).  Biases are tracked tiles, not float
          consts.  ~32 us.
    DVE   x1 = u * t (2-input ops always run 1x on real HW, so this is
          the one expensive DVE op) plus two cheap is_ge mask passes
          whose accum_out yields
            E1 = #(x1 >= ln .5) = N0 + #(t==1 & p>=.5)
            E2 = #(u  >= ln .5) = #(p >= .5)            ~34 us.
    Pool  x0 = v * t (GpSimd runs tensor_tensor mult at ~0.42 of
          roofline; STT is not supported on Pool).       ~34 us.
    PE    ones^T @ x1 / x0 accumulate column sums into two PSUM banks
          (32 matmuls each), folded to per-partition scalars at the
          end.                                           ~14 us.

  sum(t) is exact and data-independent of the heavy math, so the host
  computes it during the shard step (np.sum) instead of burning a
  device pass on the int->bf16 cast a PE reduction would need.

  Host combine (f64):  A1 = sum(x1) = sum_{t==1} ln p,
    B0 = sum(v) - sum(x0) = sum_{t==0} ln(1-p),
    loss = -(1.6*A1 + 0.4*B0)/N,
    correct = 2*E1 + sum(t) - E2 - N   (m = [p>=.5]: correct per elem
    is t*m + (1-t)(1-m); E1/E2 use the same bf16 threshold so the
    rounding fuzz cancels),  acc = correct/N.

  bf16 only ever touches ln values AFTER they are computed in f32 (no
  catastrophic cancellation); numpy emulation of the full pipeline puts
  the error at ~3e-6 (loss) / ~1e-5 (acc) relative, vs the 2e-2 gate.
"""

import sys

if "/opt/trn_rl_repo" not in sys.path:
    sys.path.insert(0, "/opt/trn_rl_repo")

import numpy as np

import concourse.bass as bass
import concourse.bacc as bacc
import concourse.tile as tile
from concourse import mybir
from concourse.bass_utils import run_bass_kernel_spmd

N_CORES = 8
N = 16777216
P = 128
SHARD = N // N_CORES          # 2097152 elements per core
COLS = SHARD // P             # 16384 columns per core
# first sub-chunk split in half so ACT/DVE start ~2us earlier
SUBS = [1024, 1024] + [2048] * 7
NS = len(SUBS)
MMCOL = 512                   # matmul free-dim tile (one PSUM bank)
TH = -0.6931471805599453      # ln(0.5)

AF = mybir.ActivationFunctionType
OP = mybir.AluOpType

# acc tile columns: [0,NS) sum(v), [NS,2NS) E1, [2NS,3NS) E2,
# [3NS] sum(x1) fold, [3NS+1] sum(x0) fold
ACC_COLS = 3 * NS + 2

_NC_CACHE = None


def build_bass():
    """Build the single-core Bass program (SPMD across 8 cores)."""
    global _NC_CACHE
    if _NC_CACHE is not None:
        return _NC_CACHE

    nc = bacc.Bacc("TRN2", target_bir_lowering=False, debug=False)

    p_in = nc.dram_tensor("p_in", [SHARD], mybir.dt.float32, kind="ExternalInput").ap()
    t_in = nc.dram_tensor("t_in", [SHARD], mybir.dt.int32, kind="ExternalInput").ap()
    acc_out = nc.dram_tensor("acc_out", [P, ACC_COLS], mybir.dt.float32, kind="ExternalOutput").ap()

    n_mm = COLS // MMCOL          # 32 matmuls per reduced quantity

    with tile.TileContext(nc) as tc:
        with (
            tc.tile_pool(name="io", bufs=NS - 1) as io_pool,
            tc.tile_pool(name="lg", bufs=3) as lg_pool,
            tc.tile_pool(name="pr", bufs=3) as pr_pool,
            tc.tile_pool(name="jk", bufs=1) as jk_pool,
            tc.tile_pool(name="ps", bufs=1, space=bass.MemorySpace.PSUM) as psum_pool,
            tc.tile_pool(name="mi", bufs=1) as misc_pool,
        ):
            ones16 = misc_pool.tile([P, P], mybir.dt.bfloat16, tag="ones16")
            nc.gpsimd.memset(ones16[:], 1.0)
            cz = misc_pool.tile([P, 1], mybir.dt.float32, tag="cz")
            co = misc_pool.tile([P, 1], mybir.dt.float32, tag="co")
            nc.gpsimd.memset(cz[:], 0.0)
            nc.gpsimd.memset(co[:], 1.0)
            warm = misc_pool.tile([P, 1], mybir.dt.float32, tag="warm")
            acc = misc_pool.tile([P, ACC_COLS], mybir.dt.float32, tag="acc")
            junk512 = misc_pool.tile([P, MMCOL], mybir.dt.float32, tag="junk512")
            psA = psum_pool.tile([P, MMCOL], mybir.dt.float32, tag="psA")
            psB = psum_pool.tile([P, MMCOL], mybir.dt.float32, tag="psB")

            # load the Ln table while the first DMA is in flight
            nc.scalar.activation(warm[:, 0:1], co[:, 0:1], AF.Ln, bias=cz[:, 0:1])

            mmA = mmB = 0
            off = 0
            for s, C in enumerate(SUBS):
                p_t = io_pool.tile([P, C], mybir.dt.float32, tag="p")
                t_t = io_pool.tile([P, C], mybir.dt.int32, tag="t")
                nc.sync.dma_start(p_t[:], p_in[off : off + C * P].rearrange("(p f) -> p f", p=P))
                nc.sync.dma_start(t_t[:], t_in[off : off + C * P].rearrange("(p f) -> p f", p=P))
                off += C * P

                ub = lg_pool.tile([P, C], mybir.dt.bfloat16, tag="ub")
                vb = lg_pool.tile([P, C], mybir.dt.bfloat16, tag="vb")
                nc.scalar.activation(ub[:], p_t[:], AF.Ln, bias=cz[:, 0:1])
                nc.scalar.activation(vb[:], p_t[:], AF.Ln, bias=co[:, 0:1], scale=-1.0,
                                     accum_out=acc[:, s : s + 1])

                x1 = pr_pool.tile([P, C], mybir.dt.bfloat16, tag="x1")
                x0 = pr_pool.tile([P, C], mybir.dt.bfloat16, tag="x0")
                nc.vector.tensor_tensor(x1[:], ub[:], t_t[:], OP.mult)
                nc.gpsimd.tensor_tensor(x0[:], vb[:], t_t[:], OP.mult)

                j1 = jk_pool.tile([P, C], mybir.dt.bfloat16, tag="j1")
                j2 = jk_pool.tile([P, C], mybir.dt.bfloat16, tag="j2")
                nc.vector.tensor_scalar(j1[:], x1[:], TH, None, OP.is_ge, OP.add,
                                        accum_out=acc[:, NS + s : NS + s + 1])
                nc.vector.tensor_scalar(j2[:], ub[:], TH, None, OP.is_ge, OP.add,
                                        accum_out=acc[:, 2 * NS + s : 2 * NS + s + 1])

                for j in range(C // MMCOL):
                    nc.tensor.matmul(psA[:], ones16[:], x1[:, j * MMCOL : (j + 1) * MMCOL],
                                     start=(mmA == 0), stop=(mmA == n_mm - 1))
                    mmA += 1
                    nc.tensor.matmul(psB[:], ones16[:], x0[:, j * MMCOL : (j + 1) * MMCOL],
                                     start=(mmB == 0), stop=(mmB == n_mm - 1))
                    mmB += 1

            nc.vector.tensor_scalar(junk512[:], psA[:], 1.0 / P, None, OP.mult,
                                    OP.add, accum_out=acc[:, 3 * NS : 3 * NS + 1])
            nc.vector.tensor_scalar(junk512[:], psB[:], 1.0 / P, None, OP.mult,
                                    OP.add, accum_out=acc[:, 3 * NS + 1 : 3 * NS + 2])

            nc.sync.dma_start(acc_out[:], acc[:])

    nc.finalize()
    _NC_CACHE = nc
    return nc


def make_in_maps(input, target):
    inp = np.ascontiguousarray(np.asarray(input, dtype=np.float32)).reshape(
        N_CORES, SHARD
    )
    tgt = np.ascontiguousarray(np.asarray(target, dtype=np.int32)).reshape(
        N_CORES, SHARD
    )
    return [{"p_in": inp[c], "t_in": tgt[c]} for c in range(N_CORES)]


def combine(results, n1_total):
    """Host-side unshard: reduce the 8 cores' partial sums -> (loss, acc)."""
    wsum = 0.0
    cnt = float(n1_total) - float(N)
    for r in results:
        a = np.asarray(r["acc_out"], dtype=np.float64)
        sumV = a[:, 0:NS].sum()
        E1 = a[:, NS : 2 * NS].sum()
        E2 = a[:, 2 * NS : 3 * NS].sum()
        A1 = a[:, 3 * NS].sum()
        X0 = a[:, 3 * NS + 1].sum()
        B0 = sumV - X0
        wsum += 1.6 * A1 + 0.4 * B0
        cnt += 2.0 * E1 - E2
    loss = -wsum / N
    acc = cnt / N
    return np.float32(loss), np.float32(acc)


def run_on_hw(input, target, **spmd_kwargs):
    nc = build_bass()
    in_maps = make_in_maps(input, target)
    return run_bass_kernel_spmd(nc, in_maps, list(range(N_CORES)), **spmd_kwargs)


def kernel(input, target):
    br = run_on_hw(input, target)
    n1_total = int(np.sum(np.asarray(target, dtype=np.int64)))
    return combine(br.results, n1_total)


# revision 14
# speedup vs baseline: 1.3922x; 1.3922x over previous
"""Balanced CE loss + accuracy on 8 Trainium2 NeuronCores (Bass/Tile).

Reference computation (N = 16777216 elements):
    loss = -sum(where(t==1, 1.6*log(p), 0.4*log(1-p))) / N
    acc  = mean(round(p) == t)

Strategy (data-parallel over N, no collectives).  Measured engine facts
on this HW: DVE 2-input ops (STT/tensor_tensor) always run 1x
(1.04 ns/col), 1-input tensor_scalar runs 2x (0.52 ns/col) regardless
of dtype, ACT runs 0.83 ns/col, GpSimd elementwise ops inflate
concurrent DVE ops ~2.4x (SBUF contention) so Pool is left idle.  The
irreducible HBM traffic is 8 B/elem (~41 us/core at full queue busy),
and runtime preamble costs ~6.7 us, so the target is to keep every
engine's busy-time near 43 us and fully overlapped with DMA:

  DVE  a1 = (p-1)*t and a0 = (t-1)*p (fused STT, the identity
       ln(a+1): a1+1 = p if t==1 else 1) + C1 mask for every sub-chunk
       (is_ge(a1,-.5) with accum = N0 + #(t1,p>=.5)) + C0 mask on the
       first/last sub-chunks.                             ~44 us
  ACT  Ln(a1+1), Ln(a0+1) in place with free-dim accum -> A1, B0 sums;
       C0 for the middle ~81% of columns via Sign(ln(y0)+ln2) accum
       (= #(y0>=.5) as +/-1 sum).                          ~43 us
  acc = (C1 + C0 - N)/N,  loss = -(1.6*A1 + 0.4*B0)/N.  All thresholds
       f32-exact (no bf16 in any value path).

Pipeline fixes vs the 73.7 us predecessor: io pool is deep enough that
all input DMAs issue without buffer-reuse stalls (the SP sequencer
dispatches each DIRECT2D in ~0.7 us, queues drain ~41 us, so DMA issue
must never wait on compute), the Ln activation table is pre-warmed
behind the first DMA, first and last sub-chunks are halved to shrink
pipeline fill/drain, and all activation biases are tracked tiles (no
const-AP preamble barrier).
"""

import sys

if "/opt/trn_rl_repo" not in sys.path:
    sys.path.insert(0, "/opt/trn_rl_repo")

import numpy as np

import concourse.bass as bass
import concourse.bacc as bacc
import concourse.tile as tile
from concourse import mybir
from concourse.bass_utils import run_bass_kernel_spmd

N_CORES = 8
N = 16777216
P = 128
SHARD = N // N_CORES          # 2097152 elements per core
COLS = SHARD // P             # 16384 columns per core
# half-size sub-chunks at both ends shrink pipeline fill and drain
SUBS = [1024, 1024, 2048, 2048, 2048, 2048, 2048, 2048, 1024, 1024]
NS = len(SUBS)
# C0 via ACT Sign for subs 1..7 (13312 cols = 81%), via DVE is_ge for
# subs 0, 8, 9 -> DVE ~44 us and ACT ~43 us, balanced
SIGN_SUBS = frozenset(range(1, 8))
LN2 = 0.6931471805599453

AF = mybir.ActivationFunctionType
OP = mybir.AluOpType

# acc tile columns per sub s: [s] A1, [NS+s] B0, [2NS+s] C1,
# [3NS+s] C0 (is_ge count for DVE subs, +/-1 sign sum for ACT subs)
ACC_COLS = 4 * NS

_NC_CACHE = None


def build_bass():
    """Build the single-core Bass program (SPMD across 8 cores)."""
    global _NC_CACHE
    if _NC_CACHE is not None:
        return _NC_CACHE

    nc = bacc.Bacc("TRN2", target_bir_lowering=False, debug=False)

    p_in = nc.dram_tensor("p_in", [SHARD], mybir.dt.float32, kind="ExternalInput").ap()
    t_in = nc.dram_tensor("t_in", [SHARD], mybir.dt.int32, kind="ExternalInput").ap()
    acc_out = nc.dram_tensor("acc_out", [P, ACC_COLS], mybir.dt.float32, kind="ExternalOutput").ap()

    with tile.TileContext(nc) as tc:
        with (
            tc.tile_pool(name="io", bufs=8) as io_pool,
            tc.tile_pool(name="wk", bufs=3) as wk_pool,
            tc.tile_pool(name="jk", bufs=1) as jk_pool,
            tc.tile_pool(name="mi", bufs=1) as misc_pool,
        ):
            co = misc_pool.tile([P, 1], mybir.dt.float32, tag="co")
            ln2c = misc_pool.tile([P, 1], mybir.dt.float32, tag="ln2c")
            nc.gpsimd.memset(co[:], 1.0)
            nc.gpsimd.memset(ln2c[:], LN2)
            warm = misc_pool.tile([P, 1], mybir.dt.float32, tag="warm")
            acc = misc_pool.tile([P, ACC_COLS], mybir.dt.float32, tag="acc")

            # load the Ln/Sign table while the first DMA is in flight
            nc.scalar.activation(warm[:, 0:1], co[:, 0:1], AF.Ln, bias=co[:, 0:1])

            off = 0
            for s, C in enumerate(SUBS):
                p_t = io_pool.tile([P, C], mybir.dt.float32, tag="p")
                t_t = io_pool.tile([P, C], mybir.dt.int32, tag="t")
                nc.sync.dma_start(p_t[:], p_in[off : off + C * P].rearrange("(p f) -> p f", p=P))
                nc.sync.dma_start(t_t[:], t_in[off : off + C * P].rearrange("(p f) -> p f", p=P))
                off += C * P

                a1 = wk_pool.tile([P, C], mybir.dt.float32, tag="a1")
                a0 = wk_pool.tile([P, C], mybir.dt.float32, tag="a0")
                # a1 = (p-1)*t = y1-1 ; a0 = (t-1)*p = y0-1  (ln(1)=0 masking)
                nc.vector.scalar_tensor_tensor(a1[:], p_t[:], -1.0, t_t[:], OP.add, OP.mult)
                nc.vector.scalar_tensor_tensor(a0[:], t_t[:], -1.0, p_t[:], OP.add, OP.mult)

                # C1 = #(y1 >= .5) = N0 + #(t==1 & p>=.5), f32-exact threshold
                j1 = jk_pool.tile([P, C], mybir.dt.bfloat16, tag="j1")
                nc.vector.tensor_scalar(j1[:], a1[:], -0.5, None, OP.is_ge, OP.add,
                                        accum_out=acc[:, 2 * NS + s : 2 * NS + s + 1])
                if s not in SIGN_SUBS:
                    # C0 = #(y0 >= .5) = N1 + #(t==0 & p<=.5) on DVE
                    j0 = jk_pool.tile([P, C], mybir.dt.bfloat16, tag="j0")
                    nc.vector.tensor_scalar(j0[:], a0[:], -0.5, None, OP.is_ge, OP.add,
                                            accum_out=acc[:, 3 * NS + s : 3 * NS + s + 1])

                # in-place Ln with free-dim accumulation: A1, B0 partials
                nc.scalar.activation(a1[:], a1[:], AF.Ln, bias=co[:, 0:1],
                                     accum_out=acc[:, s : s + 1])
                nc.scalar.activation(a0[:], a0[:], AF.Ln, bias=co[:, 0:1],
                                     accum_out=acc[:, NS + s : NS + s + 1])
                if s in SIGN_SUBS:
                    # post-Ln: sign(ln(y0)+ln2) = +/-1 for y0 >= .5 / < .5
                    js = jk_pool.tile([P, C], mybir.dt.bfloat16, tag="js")
                    nc.scalar.activation(js[:], a0[:], AF.Sign, bias=ln2c[:, 0:1],
                                         accum_out=acc[:, 3 * NS + s : 3 * NS + s + 1])

            nc.sync.dma_start(acc_out[:], acc[:])

    nc.finalize()
    _NC_CACHE = nc
    return nc


def make_in_maps(input, target):
    inp = np.ascontiguousarray(np.asarray(input, dtype=np.float32)).reshape(
        N_CORES, SHARD
    )
    tgt = np.ascontiguousarray(np.asarray(target, dtype=np.int32)).reshape(
        N_CORES, SHARD
    )
    return [{"p_in": inp[c], "t_in": tgt[c]} for c in range(N_CORES)]


def combine(results):
    """Host-side unshard: reduce the 8 cores' partial sums -> (loss, acc)."""
    A1 = B0 = C1 = C0 = 0.0
    sign_elems = sum(SUBS[s] for s in SIGN_SUBS) * P
    for r in results:
        a = np.asarray(r["acc_out"], dtype=np.float64)
        A1 += a[:, 0:NS].sum()
        B0 += a[:, NS : 2 * NS].sum()
        C1 += a[:, 2 * NS : 3 * NS].sum()
        for s in range(NS):
            col = a[:, 3 * NS + s].sum()
            if s in SIGN_SUBS:
                C0 += (col + SUBS[s] * P) / 2.0   # +/-1 sum -> count
            else:
                C0 += col
    loss = -(1.6 * A1 + 0.4 * B0) / N
    acc = (C1 + C0 - N) / N
    return np.float32(loss), np.float32(acc)


def run_on_hw(input, target, **spmd_kwargs):
    nc = build_bass()
    in_maps = make_in_maps(input, target)
    return run_bass_kernel_spmd(nc, in_maps, list(range(N_CORES)), **spmd_kwargs)


def kernel(input, target):
    br = run_on_hw(input, target)
    return combine(br.results)


# revision 16
# speedup vs baseline: 1.4281x; 1.0258x over previous
"""Balanced CE loss + accuracy on 8 Trainium2 NeuronCores (Bass/Tile).

Reference computation (N = 16777216 elements):
    loss = -sum(where(t==1, 1.6*log(p), 0.4*log(1-p))) / N
    acc  = mean(round(p) == t)

Strategy (data-parallel over N, no collectives).  Measured engine facts
on this HW (from perfetto traces of prior variants):
  - DVE 2-input ops (STT) always run 1x (~1.08 ns/col); 1-input
    tensor_scalar runs 2x (~0.52 ns/col); the accumulating
    tensor_scalar variant drops back to 1x, so counts are cheaper as
    plain is_ge masks reduced on the idle TensorE via ones^T matmuls.
  - ACT runs ~0.98 ns/col + 185 ns per accumulator read, so A1/B0
    accumulation is merged into per-round (2-sub) activations.
  - GpSimd elementwise work inflates concurrent DVE ops ~2.4x (SBUF
    contention) -> Pool stays idle.
  - HBM floor is 8 B/elem = ~41 us/core at 100% DMA-queue busy; the
    runtime preamble costs a fixed ~6.7 us and each dma_start costs
    ~0.7 us of serial SP dispatch, so the io pool is deep enough that
    DMA issue never waits on compute.

Per sub-chunk (the identity ln(1)=0 masks without a select):
    a1 = (p-1)*t = y1-1  ->  sum ln(y1) = sum_{t==1} ln(p)   =: A1
    a0 = (t-1)*p = y0-1  ->  sum ln(y0) = sum_{t==0} ln(1-p) =: B0
  each one fused DVE scalar_tensor_tensor (int32 t converted on read).
  ACT computes Ln in place per round with free-dim accumulation.
  Accuracy from exact f32 threshold counts, C1+C0-N:
    C1 = #(a1 >= -.5) = N0 + #(t1,p>=.5): DVE is_ge -> bf16 mask,
         partition-reduced by TensorE (ones^T @ mask) into PSUM.
    C0 = #(a0 >= -.5): ACT Sign(ln(y0)+ln2) accum for the middle 75%
         of columns (balancing ACT at ~47 us vs DVE at ~47 us), DVE
         masks for the head/tail rounds.
  Head and tail sub-chunks are small (512 cols) to shrink pipeline
  fill and drain; activation biases are tracked tiles (no const-AP
  barrier) and the Ln table is pre-warmed behind the first DMA.
"""

import sys

if "/opt/trn_rl_repo" not in sys.path:
    sys.path.insert(0, "/opt/trn_rl_repo")

import numpy as np

import concourse.bass as bass
import concourse.bacc as bacc
import concourse.tile as tile
from concourse import mybir
from concourse.bass_utils import run_bass_kernel_spmd

N_CORES = 8
N = 16777216
P = 128
SHARD = N // N_CORES          # 2097152 elements per core
COLS = SHARD // P             # 16384 columns per core
# sub-chunk column counts; small ends shrink pipeline fill/drain
SUBS = [512, 512, 2048, 2048, 2048, 2048, 2048, 2048, 2048, 512, 512]
assert sum(SUBS) == COLS
# ACT/DVE work is merged per round (groups of consecutive subs)
ROUNDS = [(0, 1), (2, 3), (4, 5), (6, 7), (8,), (9, 10)]
NR = len(ROUNDS)
# C0 via ACT Sign for rounds 1..3 (12288 cols), DVE is_ge otherwise
SIGN_ROUNDS = frozenset({1, 2, 3})
MMCOL = 512                   # matmul free-dim tile (one PSUM bank)
LN2 = 0.6931471805599453

AF = mybir.ActivationFunctionType
OP = mybir.AluOpType

# acc columns: [r] A1 round sums, [NR+r] B0, [2NR+r] sign C0 (+/-1
# sums, unused cols stay 0), [3NR] C1 fold, [3NR+1] DVE-C0 fold
ACC_COLS = 3 * NR + 2

_NC_CACHE = None


def build_bass():
    """Build the single-core Bass program (SPMD across 8 cores)."""
    global _NC_CACHE
    if _NC_CACHE is not None:
        return _NC_CACHE

    nc = bacc.Bacc("TRN2", target_bir_lowering=False, debug=False)

    p_in = nc.dram_tensor("p_in", [SHARD], mybir.dt.float32, kind="ExternalInput").ap()
    t_in = nc.dram_tensor("t_in", [SHARD], mybir.dt.int32, kind="ExternalInput").ap()
    acc_out = nc.dram_tensor("acc_out", [P, ACC_COLS], mybir.dt.float32, kind="ExternalOutput").ap()

    n_mm1 = COLS // MMCOL                                        # C1 matmuls
    dve_c0_cols = sum(SUBS[s] for r in range(NR) if r not in SIGN_ROUNDS
                      for s in ROUNDS[r])
    n_mm0 = dve_c0_cols // MMCOL                                 # C0 matmuls

    with tile.TileContext(nc) as tc:
        with (
            tc.tile_pool(name="io", bufs=7) as io_pool,
            tc.tile_pool(name="wk", bufs=2) as wk_pool,
            tc.tile_pool(name="jk", bufs=1) as jk_pool,
            tc.tile_pool(name="ps", bufs=1, space=bass.MemorySpace.PSUM) as psum_pool,
            tc.tile_pool(name="mi", bufs=1) as misc_pool,
        ):
            ones16 = misc_pool.tile([P, P], mybir.dt.bfloat16, tag="ones16")
            nc.gpsimd.memset(ones16[:], 1.0)
            co = misc_pool.tile([P, 1], mybir.dt.float32, tag="co")
            ln2c = misc_pool.tile([P, 1], mybir.dt.float32, tag="ln2c")
            nc.gpsimd.memset(co[:], 1.0)
            nc.gpsimd.memset(ln2c[:], LN2)
            warm = misc_pool.tile([P, 1], mybir.dt.float32, tag="warm")
            acc = misc_pool.tile([P, ACC_COLS], mybir.dt.float32, tag="acc")
            nc.gpsimd.memset(acc[:], 0.0)
            junk512 = misc_pool.tile([P, MMCOL], mybir.dt.float32, tag="junk512")
            ps1 = psum_pool.tile([P, MMCOL], mybir.dt.float32, tag="ps1")
            ps0 = psum_pool.tile([P, MMCOL], mybir.dt.float32, tag="ps0")

            # load the Ln/Sign table while the first DMA is in flight
            nc.scalar.activation(warm[:, 0:1], co[:, 0:1], AF.Ln, bias=co[:, 0:1])

            mm1 = mm0 = 0
            off = 0
            for r, subs in enumerate(ROUNDS):
                rc = sum(SUBS[s] for s in subs)
                a1 = wk_pool.tile([P, rc], mybir.dt.float32, tag="a1")
                a0 = wk_pool.tile([P, rc], mybir.dt.float32, tag="a0")
                col = 0
                for s in subs:
                    C = SUBS[s]
                    p_t = io_pool.tile([P, C], mybir.dt.float32, tag="p")
                    t_t = io_pool.tile([P, C], mybir.dt.int32, tag="t")
                    nc.sync.dma_start(p_t[:], p_in[off : off + C * P].rearrange("(p f) -> p f", p=P))
                    nc.sync.dma_start(t_t[:], t_in[off : off + C * P].rearrange("(p f) -> p f", p=P))
                    off += C * P
                    sl = slice(col, col + C)
                    # a1 = (p-1)*t ; a0 = (t-1)*p
                    nc.vector.scalar_tensor_tensor(a1[:, sl], p_t[:], -1.0, t_t[:], OP.add, OP.mult)
                    nc.vector.scalar_tensor_tensor(a0[:, sl], t_t[:], -1.0, p_t[:], OP.add, OP.mult)
                    col += C

                # C1 mask (a1 >= -.5) at 2x into bf16, reduced on TensorE
                j1 = jk_pool.tile([P, rc], mybir.dt.bfloat16, tag="j1")
                nc.vector.tensor_scalar(j1[:], a1[:], -0.5, None, OP.is_ge)
                for j in range(rc // MMCOL):
                    nc.tensor.matmul(ps1[:], ones16[:], j1[:, j * MMCOL : (j + 1) * MMCOL],
                                     start=(mm1 == 0), stop=(mm1 == n_mm1 - 1))
                    mm1 += 1
                if r not in SIGN_ROUNDS:
                    jc = jk_pool.tile([P, rc], mybir.dt.bfloat16, tag="jc")
                    nc.vector.tensor_scalar(jc[:], a0[:], -0.5, None, OP.is_ge)
                    for j in range(rc // MMCOL):
                        nc.tensor.matmul(ps0[:], ones16[:], jc[:, j * MMCOL : (j + 1) * MMCOL],
                                         start=(mm0 == 0), stop=(mm0 == n_mm0 - 1))
                        mm0 += 1

                # in-place Ln with fused free-dim accumulation
                nc.scalar.activation(a1[:], a1[:], AF.Ln, bias=co[:, 0:1],
                                     accum_out=acc[:, r : r + 1])
                nc.scalar.activation(a0[:], a0[:], AF.Ln, bias=co[:, 0:1],
                                     accum_out=acc[:, NR + r : NR + r + 1])
                if r in SIGN_ROUNDS:
                    # post-Ln: sign(ln(y0)+ln2) = +/-1 for y0 >= .5 / < .5
                    jc = jk_pool.tile([P, rc], mybir.dt.bfloat16, tag="jc")
                    nc.scalar.activation(jc[:], a0[:], AF.Sign, bias=ln2c[:, 0:1],
                                         accum_out=acc[:, 2 * NR + r : 2 * NR + r + 1])

            # fold the PSUM count matrices (128 identical rows) into columns
            nc.vector.tensor_scalar(junk512[:], ps1[:], 1.0 / P, None, OP.mult,
                                    OP.add, accum_out=acc[:, 3 * NR : 3 * NR + 1])
            nc.vector.tensor_scalar(junk512[:], ps0[:], 1.0 / P, None, OP.mult,
                                    OP.add, accum_out=acc[:, 3 * NR + 1 : 3 * NR + 2])

            nc.sync.dma_start(acc_out[:], acc[:])

    nc.finalize()
    _NC_CACHE = nc
    return nc


def make_in_maps(input, target):
    inp = np.ascontiguousarray(np.asarray(input, dtype=np.float32)).reshape(
        N_CORES, SHARD
    )
    tgt = np.ascontiguousarray(np.asarray(target, dtype=np.int32)).reshape(
        N_CORES, SHARD
    )
    return [{"p_in": inp[c], "t_in": tgt[c]} for c in range(N_CORES)]


def combine(results):
    """Host-side unshard: reduce the 8 cores' partial sums -> (loss, acc)."""
    A1 = B0 = S0 = C1 = C0m = 0.0
    sign_elems = sum(SUBS[s] for r in SIGN_ROUNDS for s in ROUNDS[r]) * P
    for r in results:
        a = np.asarray(r["acc_out"], dtype=np.float64)
        A1 += a[:, 0:NR].sum()
        B0 += a[:, NR : 2 * NR].sum()
        S0 += a[:, 2 * NR : 3 * NR].sum()
        C1 += a[:, 3 * NR].sum()
        C0m += a[:, 3 * NR + 1].sum()
    loss = -(1.6 * A1 + 0.4 * B0) / N
    C0 = (S0 + N_CORES * sign_elems) / 2.0 + C0m
    acc = (C1 + C0 - N) / N
    return np.float32(loss), np.float32(acc)


def run_on_hw(input, target, **spmd_kwargs):
    nc = build_bass()
    in_maps = make_in_maps(input, target)
    return run_bass_kernel_spmd(nc, in_maps, list(range(N_CORES)), **spmd_kwargs)


def kernel(input, target):
    br = run_on_hw(input, target)
    return combine(br.results)
